# revision 9
# baseline (speedup 1.0000x reference)
"""MoE router kernel for Trainium2 (Bass/Tile), SPMD across 8 NeuronCores.

Problem: nn_MoERouter (B=8, T=4096, D=1024, E=64, TOP_K=2).

  router_logits = (x * mask) @ W.T * mask          # (B, T, E)
  router_probs  = softmax(router_logits) * mask
  expert_weights, expert_indices = top_k(probs, 2), renormalized, masked
  padded tokens get index -1

Sharding: data-parallel over the batch axis; core b handles x[b] (4096
tokens). W is tiny and replicated. No cross-core communication.

Layout strategy (per core):
  - The matmul contracts over d, so d must live on SBUF partitions for both
    operands. x arrives [T, D] row-major; fp32 can't use the xbar DMA
    transpose (2-byte only), so the host hands the kernel x[b].T ([D, T],
    contiguous) and W.T. All device DMAs are then dense.
  - Per 128-token tile: 8 accumulating matmuls (lhsT = x.T chunk [128d,
    128tok] stationary, rhs = W.T chunk [128d, 64e]) -> PSUM logits
    [128 tok, 64 exp]: softmax/top-k then reduce along the free dim.
  - ACT: masked-logits copy (PSUM->SBUF) + exp with fused per-token sum.
  - DVE: reciprocal, probs scale, top-8 + indices (InstMax/InstMaxIndex).
  - Tail (batched over all 32 tiles): top-2 renormalize, index masking.
"""

import os
import sys

import numpy as np

for _p in ("/opt/trn_rl_repo", "/opt/pypackages"):
    if _p not in sys.path and os.path.isdir(_p):
        sys.path.append(_p)

import concourse.bass as bass
import concourse.mybir as mybir
from concourse import bacc
from concourse.tile import TileContext

F32 = mybir.dt.float32
I32 = mybir.dt.int32
U32 = mybir.dt.uint32

B, T, D, E, TOP_K = 8, 4096, 1024, 64, 2
N_CORES = 8
P = 128                    # SBUF partitions
D_CHUNKS = D // P          # 8 contraction chunks
TOK_BLK = 512              # tokens per DMA block
SUBS = TOK_BLK // P        # 4 token tiles per block


def build_moe_router(t_core: int = T) -> bacc.Bacc:
    """Build the per-core Bass program. t_core tokens per core (multiple of 512)."""
    assert t_core % TOK_BLK == 0
    n_blk = t_core // TOK_BLK
    n_tiles = t_core // P

    nc = bacc.Bacc("TRN2", target_bir_lowering=False, debug=False)

    xT = nc.dram_tensor("xT", [D, t_core], F32, kind="ExternalInput")
    wT = nc.dram_tensor("wT", [D, E], F32, kind="ExternalInput")
    maskf = nc.dram_tensor("maskf", [P, n_tiles], F32, kind="ExternalInput")
    logits_d = nc.dram_tensor("logits", [t_core, E], F32, kind="ExternalOutput")
    probs_d = nc.dram_tensor("probs", [t_core, E], F32, kind="ExternalOutput")
    weights_d = nc.dram_tensor("weights", [t_core, TOP_K], F32, kind="ExternalOutput")
    indices_d = nc.dram_tensor("indices", [t_core, TOP_K], I32, kind="ExternalOutput")

    xT_t = xT.rearrange("(c p) t -> p c t", p=P)          # [128, 8, t_core]
    wT_t = wT.rearrange("(c p) e -> p c e", p=P)          # [128, 8, 64]
    logits_t = logits_d.rearrange("(n s p) e -> p n s e", p=P, s=SUBS)
    probs_t = probs_d.rearrange("(n s p) e -> p n s e", p=P, s=SUBS)
    weights_t = weights_d.rearrange("(i p) k -> p i k", p=P)
    indices_t = indices_d.rearrange("(i p) k -> p i k", p=P)

    with TileContext(nc) as tc:
        with (
            tc.tile_pool(name="xpool", bufs=3) as xpool,
            tc.tile_pool(name="consts", bufs=1) as consts,
            tc.tile_pool(name="psum", bufs=6, space="PSUM") as psum_pool,
            tc.tile_pool(name="psink", bufs=1, space="PSUM") as psink_pool,
            tc.tile_pool(name="stage", bufs=3) as stage,
            tc.tile_pool(name="small", bufs=6) as small,
            tc.tile_pool(name="accs", bufs=1) as accs,
        ):
            w_sb = consts.tile([P, D_CHUNKS, E], F32)
            nc.sync.dma_start(out=w_sb, in_=wT_t)
            maskf_sb = consts.tile([P, n_tiles], F32)
            nc.sync.dma_start(out=maskf_sb, in_=maskf[:, :])

            top8 = accs.tile([P, n_tiles, 8], F32)
            idx8 = accs.tile([P, n_tiles, 8], U32)

            # Walrus allows only one sync-wait on a (fp32) Matmult. This PE
            # sink matmul absorbs DMA-completion waits so the real matmuls
            # after it carry at most one wait each.
            sink_ps = psink_pool.tile([1, 1], F32)
            nc.tensor.matmul(sink_ps, lhsT=w_sb[:, 0, 0:1], rhs=w_sb[:, 0, 0:1])

            for blk in range(n_blk):
                x_sb = xpool.tile([P, D_CHUNKS, TOK_BLK], F32)
                nc.sync.dma_start(
                    out=x_sb, in_=xT_t[:, :, blk * TOK_BLK : (blk + 1) * TOK_BLK]
                )
                # absorb this block's x-DMA wait on PE (see sink_ps above)
                nc.tensor.matmul(sink_ps, lhsT=x_sb[:, 0, 0:1], rhs=x_sb[:, 0, 0:1])
                lg_stage = stage.tile([P, SUBS, E], F32)
                pr_stage = stage.tile([P, SUBS, E], F32)

                for sub in range(SUBS):
                    i = blk * SUBS + sub  # 128-token tile index
                    lg_ps = psum_pool.tile([P, E], F32)
                    for c in range(D_CHUNKS):
                        nc.tensor.matmul(
                            lg_ps,
                            lhsT=x_sb[:, c, sub * P : (sub + 1) * P],
                            rhs=w_sb[:, c, :],
                            start=(c == 0),
                            stop=(c == D_CHUNKS - 1),
                        )
                    m_col = maskf_sb[:, i : i + 1]
                    # masked logits PSUM -> SBUF (out = in * mask)
                    nc.scalar.mul(lg_stage[:, sub, :], lg_ps, m_col)
                    # exps = exp(logits * mask), fused per-token sum
                    exp_t = small.tile([P, E], F32)
                    sum_t = small.tile([P, 1], F32)
                    nc.scalar.activation(
                        out=exp_t,
                        in_=lg_ps,
                        func=mybir.ActivationFunctionType.Exp,
                        scale=m_col,
                        accum_out=sum_t,
                    )
                    # probs = exps * (mask / sum)
                    r_t = small.tile([P, 1], F32)
                    nc.vector.reciprocal(r_t, sum_t)
                    r2_t = small.tile([P, 1], F32)
                    nc.vector.tensor_mul(r2_t, r_t, m_col)
                    nc.vector.tensor_scalar_mul(pr_stage[:, sub, :], exp_t, r2_t)
                    # top-8 values + indices (ordering matches top-k of probs)
                    nc.vector.max(out=top8[:, i, :], in_=exp_t)
                    nc.vector.max_index(
                        out=idx8[:, i, :], in_max=top8[:, i, :], in_values=exp_t
                    )

                nc.sync.dma_start(out=logits_t[:, blk, :, :], in_=lg_stage)
                nc.sync.dma_start(out=probs_t[:, blk, :, :], in_=pr_stage)

            # ---- tail: renormalized top-2 weights + masked indices ----
            s_t = accs.tile([P, n_tiles], F32)
            nc.vector.tensor_add(s_t, top8[:, :, 0], top8[:, :, 1])
            rs_t = accs.tile([P, n_tiles], F32)
            nc.vector.reciprocal(rs_t, s_t)
            nc.vector.tensor_mul(rs_t, rs_t, maskf_sb)
            w_out = accs.tile([P, n_tiles, TOP_K], F32)
            for k in range(TOP_K):
                nc.vector.tensor_mul(w_out[:, :, k], top8[:, :, k], rs_t)
            nc.sync.dma_start(out=weights_t, in_=w_out)

            # indices: (idx + 1) * mask - 1  (exact in fp32)
            idxf = accs.tile([P, n_tiles, TOP_K], F32)
            nc.vector.tensor_copy(idxf, idx8[:, :, 0:TOP_K])
            for k in range(TOP_K):
                nc.vector.tensor_scalar_add(idxf[:, :, k], idxf[:, :, k], 1.0)
                nc.vector.tensor_mul(idxf[:, :, k], idxf[:, :, k], maskf_sb)
                nc.vector.tensor_scalar_add(idxf[:, :, k], idxf[:, :, k], -1.0)
            idxi = accs.tile([P, n_tiles, TOP_K], I32)
            nc.vector.tensor_copy(idxi, idxf)
            nc.sync.dma_start(out=indices_t, in_=idxi)

    # Legalization (splits >1-wait instructions into event-semaphore ops,
    # moves matmul waits to ldweights) — required by walrus codegen.
    nc.compile()
    return nc


_NC_CACHE: dict[int, bass.Bass] = {}


def _get_nc(t_core: int = T) -> bass.Bass:
    if t_core not in _NC_CACHE:
        _NC_CACHE[t_core] = build_moe_router(t_core)
    return _NC_CACHE[t_core]


def make_in_maps(x: np.ndarray, x_mask: np.ndarray, W: np.ndarray):
    """Shard full inputs into per-core input maps (host-side layout prep)."""
    n_tiles = x.shape[1] // P
    wt = np.ascontiguousarray(np.asarray(W, dtype=np.float32).T)  # [D, E]
    in_maps = []
    for b in range(x.shape[0]):
        xt = np.ascontiguousarray(np.asarray(x[b], dtype=np.float32).T)  # [D, T]
        mf = np.ascontiguousarray(
            np.asarray(x_mask[b], dtype=np.float32).reshape(n_tiles, P).T
        )
        in_maps.append({"xT": xt, "wT": wt, "maskf": mf})
    return in_maps


def run_kernel(x, x_mask, W, trace: bool = False, trace_kwargs: dict | None = None):
    """Run on hardware; returns (outputs_tuple, BassKernelResults)."""
    from concourse.bass_utils import run_bass_kernel_spmd

    x = np.asarray(x)
    x_mask = np.asarray(x_mask)
    W = np.asarray(W)
    n_cores = x.shape[0]
    nc = _get_nc(x.shape[1])
    in_maps = make_in_maps(x, x_mask, W)
    res = run_bass_kernel_spmd(
        nc,
        in_maps,
        core_ids=list(range(n_cores)),
        trace=trace,
        **(trace_kwargs or {}),
    )
    ew = np.stack([res.results[b]["weights"] for b in range(n_cores)])
    ei = np.stack([res.results[b]["indices"] for b in range(n_cores)])
    rl = np.stack([res.results[b]["logits"] for b in range(n_cores)])
    rp = np.stack([res.results[b]["probs"] for b in range(n_cores)])
    return (ew, ei, rl, rp), res


def kernel(**inputs):
    outs, _ = run_kernel(
        inputs["x"], inputs["x_mask"], inputs["W"],
        trace=os.environ.get("MOE_TRACE", "") == "1",
    )
    return outs


# revision 11
# speedup vs baseline: 1.7392x; 1.7392x over previous
"""MoE router kernel for Trainium2 (Bass/Tile), SPMD across 8 NeuronCores.

Problem: nn_MoERouter (B=8, T=4096, D=1024, E=64, TOP_K=2).

  router_logits = (x * mask) @ W.T * mask          # (B, T, E)
  router_probs  = softmax(router_logits) * mask
  expert_weights, expert_indices = top_k(probs, 2), renormalized, masked
  padded tokens get index -1

Sharding: data-parallel over the batch axis; core b handles x[b] (4096
tokens). W is tiny and replicated. No cross-core communication.

Matmul strategy (per core): plain fp32 matmul on TRN2 lowers to 2
half-rate passes (~8 ns/output-col measured) and float32r is only
~tf32-accurate (1.5e-4 — would flip near-tie expert indices). Instead we
use an error-compensated fp16 split computed on the host:

  x = xh + xls/2048,  W = Wh + Wls/2048   (xh/xls/Wh/Wls all fp16)
  logits = xh@Wh + (xh@Wls + xls@Wh)/2048    (drops xls@Wls ~ 2^-22)

Max logits error ~4e-6 (verified on the real inputs: 0/65536 index
flips), DMA volume unchanged (2+2 bytes/elem), and the matmuls run at
full 1 cycle/row rate. The contraction (d) must be on SBUF partitions,
so the host also hands the kernel x[b].T — all device DMAs are dense.

Layout: W-stationary, N=512 tokens per matmul, out = logits.T [64, 512]
in PSUM. The main term accumulates in PSUM partitions 0:64 and both
correction terms in 64:128 (concurrent column-packed PE tiles), then a
single DVE scalar_tensor_tensor combines halves: lgT = B/2048 + A.
Four PE transposes per block bring logits back to [128 tok, 64 exp],
where ACT does a batched exp and DVE does softmax scaling + top-8 with
indices (InstMax/InstMaxIndex match jax.lax.top_k tie order).
"""

import os
import sys

import numpy as np

for _p in ("/opt/trn_rl_repo", "/opt/pypackages"):
    if _p not in sys.path and os.path.isdir(_p):
        sys.path.append(_p)

import concourse.bass as bass
import concourse.mybir as mybir
from concourse import bacc
from concourse.masks import make_identity
from concourse.tile import TileContext

F32 = mybir.dt.float32
F16 = mybir.dt.float16
I32 = mybir.dt.int32
U32 = mybir.dt.uint32

B, T, D, E, TOP_K = 8, 4096, 1024, 64, 2
N_CORES = 8
P = 128                    # SBUF partitions
D_CHUNKS = D // P          # 8 contraction chunks
TOK_BLK = 512              # tokens per block (matmul free dim)
SUBS = TOK_BLK // P        # 4 token tiles per block
SPLIT_SCALE = 2048.0       # 2^11 residual scale for the fp16 split


def _bcast(ap: bass.AP, n: int) -> bass.AP:
    """Append a step-0 dim of size n (free-dim broadcast for DVE reads)."""
    return bass.AP(tensor=ap.tensor, offset=ap.offset, ap=[*ap.ap, [0, n]])


def build_moe_router(t_core: int = T) -> bacc.Bacc:
    """Build the per-core Bass program. t_core tokens per core (mult of 512)."""
    assert t_core % TOK_BLK == 0
    n_blk = t_core // TOK_BLK
    n_tiles = t_core // P

    nc = bacc.Bacc("TRN2", target_bir_lowering=False, debug=False)

    xhT = nc.dram_tensor("xhT", [D, t_core], F16, kind="ExternalInput")
    xlT = nc.dram_tensor("xlT", [D, t_core], F16, kind="ExternalInput")
    whT = nc.dram_tensor("whT", [D, E], F16, kind="ExternalInput")
    wlT = nc.dram_tensor("wlT", [D, E], F16, kind="ExternalInput")
    maskf = nc.dram_tensor("maskf", [P, n_tiles], F32, kind="ExternalInput")
    logits_d = nc.dram_tensor("logits", [t_core, E], F32, kind="ExternalOutput")
    probs_d = nc.dram_tensor("probs", [t_core, E], F32, kind="ExternalOutput")
    weights_d = nc.dram_tensor("weights", [t_core, TOP_K], F32, kind="ExternalOutput")
    indices_d = nc.dram_tensor("indices", [t_core, TOP_K], I32, kind="ExternalOutput")

    xhT_t = xhT.rearrange("(c p) t -> p c t", p=P)        # [128, 8, t_core]
    xlT_t = xlT.rearrange("(c p) t -> p c t", p=P)
    whT_t = whT.rearrange("(c p) e -> p c e", p=P)        # [128, 8, 64]
    wlT_t = wlT.rearrange("(c p) e -> p c e", p=P)
    logits_t = logits_d.rearrange("(n s p) e -> p n s e", p=P, s=SUBS)
    probs_t = probs_d.rearrange("(n s p) e -> p n s e", p=P, s=SUBS)
    weights_t = weights_d.rearrange("(i p) k -> p i k", p=P)
    indices_t = indices_d.rearrange("(i p) k -> p i k", p=P)

    MUL = mybir.AluOpType.mult
    ADD = mybir.AluOpType.add

    with TileContext(nc) as tc:
        with (
            tc.tile_pool(name="xpool", bufs=4) as xpool,
            tc.tile_pool(name="consts", bufs=1) as consts,
            tc.tile_pool(name="psT", bufs=2, space="PSUM") as psT,
            tc.tile_pool(name="psL", bufs=2, space="PSUM") as psL,
            tc.tile_pool(name="psink", bufs=1, space="PSUM") as psink,
            tc.tile_pool(name="stage", bufs=3) as stage,
            tc.tile_pool(name="small", bufs=6) as small,
            tc.tile_pool(name="accs", bufs=1) as accs,
        ):
            wh_sb = consts.tile([P, D_CHUNKS, E], F16)
            wl_sb = consts.tile([P, D_CHUNKS, E], F16)
            nc.sync.dma_start(out=wh_sb, in_=whT_t)
            nc.sync.dma_start(out=wl_sb, in_=wlT_t)
            maskf_sb = consts.tile([P, n_tiles], F32)
            nc.sync.dma_start(out=maskf_sb, in_=maskf[:, :])
            ident = consts.tile([E, E], F32)
            make_identity(nc, ident)

            top8 = accs.tile([P, n_tiles, 8], F32)
            idx8 = accs.tile([P, n_tiles, 8], U32)

            # PE sink matmuls absorb DMA-completion waits so real matmuls
            # carry at most one wait (walrus limit on Matmult sync waits).
            sink_ps = psink.tile([1, 1], F32)
            nc.tensor.matmul(sink_ps, lhsT=wh_sb[:, 0, 0:1], rhs=wh_sb[:, 0, 0:1])
            nc.tensor.matmul(sink_ps, lhsT=wl_sb[:, 0, 0:1], rhs=wl_sb[:, 0, 0:1])

            for blk in range(n_blk):
                tsl = slice(blk * TOK_BLK, (blk + 1) * TOK_BLK)
                xh_sb = xpool.tile([P, D_CHUNKS, TOK_BLK], F16)
                xl_sb = xpool.tile([P, D_CHUNKS, TOK_BLK], F16)
                nc.sync.dma_start(out=xh_sb, in_=xhT_t[:, :, tsl])
                nc.sync.dma_start(out=xl_sb, in_=xlT_t[:, :, tsl])
                nc.tensor.matmul(sink_ps, lhsT=xh_sb[:, 0, 0:1], rhs=xh_sb[:, 0, 0:1])
                nc.tensor.matmul(sink_ps, lhsT=xl_sb[:, 0, 0:1], rhs=xl_sb[:, 0, 0:1])

                # logits.T: main term -> PSUM partitions 0:64, correction
                # terms (x2048) -> 64:128; the two column tiles run
                # concurrently on the PE array.
                lgT_ps = psT.tile([P, TOK_BLK], F32)
                for c in range(D_CHUNKS):
                    nc.tensor.matmul(
                        lgT_ps[0:E, :], lhsT=wh_sb[:, c, :], rhs=xh_sb[:, c, :],
                        start=(c == 0), stop=(c == D_CHUNKS - 1),
                        skip_group_check=True,
                    )
                    nc.tensor.matmul(
                        lgT_ps[E : 2 * E, :], lhsT=wl_sb[:, c, :], rhs=xh_sb[:, c, :],
                        start=(c == 0), stop=False, skip_group_check=True,
                    )
                    nc.tensor.matmul(
                        lgT_ps[E : 2 * E, :], lhsT=wh_sb[:, c, :], rhs=xl_sb[:, c, :],
                        start=False, stop=(c == D_CHUNKS - 1), skip_group_check=True,
                    )

                # lgT = correction/2048 + main   [64, 512] fp32 in SBUF
                # (DVE has one PSUM read port: ACT moves the main half to
                # SBUF, DVE adds the scaled correction from PSUM onto it.)
                lgT_sb = stage.tile([E, TOK_BLK], F32)
                nc.scalar.copy(lgT_sb, lgT_ps[0:E, :])
                nc.vector.scalar_tensor_tensor(
                    out=lgT_sb, in0=lgT_ps[E : 2 * E, :], scalar=1.0 / SPLIT_SCALE,
                    in1=lgT_sb, op0=MUL, op1=ADD,
                )

                # transpose back to [128 tok, 64 exp] tiles (PSUM, one bank)
                lg_ps = psL.tile([P, SUBS, E], F32)
                for sub in range(SUBS):
                    nc.tensor.matmul(
                        lg_ps[:, sub, :], lhsT=lgT_sb[:, sub * P : (sub + 1) * P],
                        rhs=ident, is_transpose=True, skip_group_check=True,
                    )

                mask_blk = maskf_sb[:, blk * SUBS : (blk + 1) * SUBS]  # [128, 4]

                # masked logits PSUM -> SBUF (also the DMA staging buffer)
                lg_sb = stage.tile([P, SUBS, E], F32)
                nc.vector.tensor_mul(lg_sb, lg_ps, _bcast(mask_blk, E))
                nc.sync.dma_start(out=logits_t[:, blk, :, :], in_=lg_sb)

                # exps (unmasked is fine: masked rows are overridden later)
                exp_sb = stage.tile([P, SUBS, E], F32)
                nc.scalar.activation(
                    out=exp_sb, in_=lg_ps, func=mybir.ActivationFunctionType.Exp
                )

                sums = small.tile([P, SUBS, 1], F32)
                nc.vector.reduce_sum(sums, exp_sb, axis=mybir.AxisListType.X)
                r_t = small.tile([P, SUBS], F32)
                nc.vector.reciprocal(r_t, sums[:, :, 0])
                r2_t = small.tile([P, SUBS], F32)
                nc.vector.tensor_mul(r2_t, r_t, mask_blk)
                pr_sb = stage.tile([P, SUBS, E], F32)
                nc.vector.tensor_mul(pr_sb, exp_sb, _bcast(r2_t[:, :], E))
                nc.sync.dma_start(out=probs_t[:, blk, :, :], in_=pr_sb)

                for sub in range(SUBS):
                    i = blk * SUBS + sub
                    nc.vector.max(out=top8[:, i, :], in_=exp_sb[:, sub, :])
                    nc.vector.max_index(
                        out=idx8[:, i, :], in_max=top8[:, i, :],
                        in_values=exp_sb[:, sub, :],
                    )

            # ---- tail: renormalized top-2 weights + masked indices ----
            s_t = accs.tile([P, n_tiles], F32)
            nc.vector.tensor_add(s_t, top8[:, :, 0], top8[:, :, 1])
            rs_t = accs.tile([P, n_tiles], F32)
            nc.vector.reciprocal(rs_t, s_t)
            nc.vector.tensor_mul(rs_t, rs_t, maskf_sb)
            w_out = accs.tile([P, n_tiles, TOP_K], F32)
            for k in range(TOP_K):
                nc.vector.tensor_mul(w_out[:, :, k], top8[:, :, k], rs_t)
            nc.sync.dma_start(out=weights_t, in_=w_out)

            # indices: (idx + 1) * mask - 1  (exact in fp32)
            idxf = accs.tile([P, n_tiles, TOP_K], F32)
            nc.vector.tensor_copy(idxf, idx8[:, :, 0:TOP_K])
            for k in range(TOP_K):
                nc.vector.tensor_scalar_add(idxf[:, :, k], idxf[:, :, k], 1.0)
                nc.vector.tensor_mul(idxf[:, :, k], idxf[:, :, k], maskf_sb)
                nc.vector.tensor_scalar_add(idxf[:, :, k], idxf[:, :, k], -1.0)
            idxi = accs.tile([P, n_tiles, TOP_K], I32)
            nc.vector.tensor_copy(idxi, idxf)
            nc.sync.dma_start(out=indices_t, in_=idxi)

    # Legalization (splits >1-wait instructions into event-semaphore ops,
    # moves matmul waits to ldweights) — required by walrus codegen.
    nc.compile()
    return nc


_NC_CACHE: dict[int, bacc.Bacc] = {}


def _get_nc(t_core: int = T) -> bacc.Bacc:
    if t_core not in _NC_CACHE:
        _NC_CACHE[t_core] = build_moe_router(t_core)
    return _NC_CACHE[t_core]


def _split16(a: np.ndarray):
    hi = a.astype(np.float16)
    lo = ((a - hi.astype(np.float32)) * SPLIT_SCALE).astype(np.float16)
    return hi, lo


def make_in_maps(x: np.ndarray, x_mask: np.ndarray, W: np.ndarray):
    """Shard full inputs into per-core input maps (host-side layout prep)."""
    n_tiles = x.shape[1] // P
    wh, wl = _split16(np.asarray(W, dtype=np.float32))
    whT = np.ascontiguousarray(wh.T)
    wlT = np.ascontiguousarray(wl.T)
    in_maps = []
    for b in range(x.shape[0]):
        xh, xl = _split16(np.asarray(x[b], dtype=np.float32))
        mf = np.ascontiguousarray(
            np.asarray(x_mask[b], dtype=np.float32).reshape(n_tiles, P).T
        )
        in_maps.append(
            {
                "xhT": np.ascontiguousarray(xh.T),
                "xlT": np.ascontiguousarray(xl.T),
                "whT": whT,
                "wlT": wlT,
                "maskf": mf,
            }
        )
    return in_maps


def run_kernel(x, x_mask, W, trace: bool = False, trace_kwargs: dict | None = None):
    """Run on hardware; returns (outputs_tuple, BassKernelResults)."""
    from concourse.bass_utils import run_bass_kernel_spmd

    x = np.asarray(x)
    x_mask = np.asarray(x_mask)
    W = np.asarray(W)
    n_cores = x.shape[0]
    nc = _get_nc(x.shape[1])
    in_maps = make_in_maps(x, x_mask, W)
    res = run_bass_kernel_spmd(
        nc,
        in_maps,
        core_ids=list(range(n_cores)),
        trace=trace,
        **(trace_kwargs or {}),
    )
    ew = np.stack([res.results[b]["weights"] for b in range(n_cores)])
    ei = np.stack([res.results[b]["indices"] for b in range(n_cores)])
    rl = np.stack([res.results[b]["logits"] for b in range(n_cores)])
    rp = np.stack([res.results[b]["probs"] for b in range(n_cores)])
    return (ew, ei, rl, rp), res


def kernel(**inputs):
    outs, _ = run_kernel(
        inputs["x"], inputs["x_mask"], inputs["W"],
        trace=os.environ.get("MOE_TRACE", "") == "1",
    )
    return outs


# revision 12
# speedup vs baseline: 2.1095x; 1.2129x over previous
"""MoE router kernel for Trainium2 (Bass/Tile), SPMD across 8 NeuronCores.

Problem: nn_MoERouter (B=8, T=4096, D=1024, E=64, TOP_K=2).

  router_logits = (x * mask) @ W.T * mask          # (B, T, E)
  router_probs  = softmax(router_logits) * mask
  expert_weights, expert_indices = top_k(probs, 2), renormalized, masked
  padded tokens get index -1

Sharding: data-parallel over the batch axis; core b handles x[b] (4096
tokens). W is tiny and replicated. No cross-core communication.

Matmul strategy (per core): plain fp32 matmul on TRN2 lowers to 2
half-rate passes (~8 ns/output-col measured) and float32r is only
~tf32-accurate (1.5e-4 — would flip near-tie expert indices). Instead we
use an error-compensated fp16 split computed on the host:

  x = xh + xls/2048,  W = Wh + Wls/2048   (xh/xls/Wh/Wls all fp16)
  logits = xh@Wh + (xh@Wls + xls@Wh)/2048    (drops xls@Wls ~ 2^-22)

Max logits error ~4e-6 (verified on the real inputs: 0/65536 index
flips), DMA volume unchanged (2+2 bytes/elem), and the matmuls run at
full 1 cycle/row rate.

Layouts: the contraction (d) must be on SBUF partitions and DMA
descriptors want long contiguous per-partition runs, so the host
pre-tiles x into the exact per-block SBUF layout
[n_blk, 128p, 8chunk, 512tok] (every DMA is a fully contiguous 1 MB
read, 8 KB per partition). logits/probs are likewise written in packed
per-block layout [n_blk, 128p, 4sub, 64e] (1 KB/partition runs) and
unpacked on the host.

Compute: W-stationary matmuls, N=512 tokens, out = logits.T [64, 512]
in PSUM. The main term accumulates in PSUM partitions 0:64 and both
correction terms in 64:128 (concurrent column-packed PE tiles), then
ACT moves the main half to SBUF and one DVE scalar_tensor_tensor adds
correction/2048 (DVE has a single PSUM read port). Four PE transposes
per block bring logits back to [128 tok, 64 exp], ACT does a batched
exp, DVE does softmax scaling + top-8 with indices
(InstMax/InstMaxIndex match jax.lax.top_k tie order).
"""

import os
import sys

import numpy as np

for _p in ("/opt/trn_rl_repo", "/opt/pypackages"):
    if _p not in sys.path and os.path.isdir(_p):
        sys.path.append(_p)

import concourse.bass as bass
import concourse.mybir as mybir
from concourse import bacc
from concourse.masks import make_identity
from concourse.tile import TileContext

F32 = mybir.dt.float32
F16 = mybir.dt.float16
I32 = mybir.dt.int32
U32 = mybir.dt.uint32

B, T, D, E, TOP_K = 8, 4096, 1024, 64, 2
N_CORES = 8
P = 128                    # SBUF partitions
D_CHUNKS = D // P          # 8 contraction chunks
TOK_BLK = 512              # tokens per block (matmul free dim)
SUBS = TOK_BLK // P        # 4 token tiles per block
SPLIT_SCALE = 2048.0       # 2^11 residual scale for the fp16 split


def _bcast(ap: bass.AP, n: int) -> bass.AP:
    """Append a step-0 dim of size n (free-dim broadcast for DVE reads)."""
    return bass.AP(tensor=ap.tensor, offset=ap.offset, ap=[*ap.ap, [0, n]])


def build_moe_router(t_core: int = T) -> bacc.Bacc:
    """Build the per-core Bass program. t_core tokens per core (mult of 512)."""
    assert t_core % TOK_BLK == 0
    n_blk = t_core // TOK_BLK
    n_tiles = t_core // P

    nc = bacc.Bacc("TRN2", target_bir_lowering=False, debug=False)

    xhP = nc.dram_tensor("xhP", [n_blk, P, D_CHUNKS, TOK_BLK], F16, kind="ExternalInput")
    xlP = nc.dram_tensor("xlP", [n_blk, P, D_CHUNKS, TOK_BLK], F16, kind="ExternalInput")
    whT = nc.dram_tensor("whT", [D, E], F16, kind="ExternalInput")
    wlT = nc.dram_tensor("wlT", [D, E], F16, kind="ExternalInput")
    maskf = nc.dram_tensor("maskf", [P, n_tiles], F32, kind="ExternalInput")
    logits_d = nc.dram_tensor("logits", [n_blk, P, SUBS, E], F32, kind="ExternalOutput")
    probs_d = nc.dram_tensor("probs", [n_blk, P, SUBS, E], F32, kind="ExternalOutput")
    weights_d = nc.dram_tensor("weights", [P, n_tiles, TOP_K], F32, kind="ExternalOutput")
    indices_d = nc.dram_tensor("indices", [P, n_tiles, TOP_K], I32, kind="ExternalOutput")

    whT_t = whT.rearrange("(c p) e -> p c e", p=P)        # [128, 8, 64]
    wlT_t = wlT.rearrange("(c p) e -> p c e", p=P)

    MUL = mybir.AluOpType.mult
    ADD = mybir.AluOpType.add

    with TileContext(nc) as tc:
        with (
            tc.tile_pool(name="xpool", bufs=4) as xpool,
            tc.tile_pool(name="consts", bufs=1) as consts,
            tc.tile_pool(name="psT", bufs=2, space="PSUM") as psT,
            tc.tile_pool(name="psL", bufs=2, space="PSUM") as psL,
            tc.tile_pool(name="psink", bufs=1, space="PSUM") as psink,
            tc.tile_pool(name="stage", bufs=3) as stage,
            tc.tile_pool(name="small", bufs=6) as small,
            tc.tile_pool(name="accs", bufs=1) as accs,
        ):
            wh_sb = consts.tile([P, D_CHUNKS, E], F16)
            wl_sb = consts.tile([P, D_CHUNKS, E], F16)
            nc.sync.dma_start(out=wh_sb, in_=whT_t)
            nc.sync.dma_start(out=wl_sb, in_=wlT_t)
            maskf_sb = consts.tile([P, n_tiles], F32)
            nc.sync.dma_start(out=maskf_sb, in_=maskf[:, :])
            ident = consts.tile([E, E], F32)
            make_identity(nc, ident)

            top8 = accs.tile([P, n_tiles, 8], F32)
            idx8 = accs.tile([P, n_tiles, 8], U32)

            # PE sink matmuls absorb DMA-completion waits so real matmuls
            # carry at most one wait (walrus limit on Matmult sync waits).
            sink_ps = psink.tile([1, 1], F32)
            nc.tensor.matmul(sink_ps, lhsT=wh_sb[:, 0, 0:1], rhs=wh_sb[:, 0, 0:1])
            nc.tensor.matmul(sink_ps, lhsT=wl_sb[:, 0, 0:1], rhs=wl_sb[:, 0, 0:1])

            for blk in range(n_blk):
                xh_sb = xpool.tile([P, D_CHUNKS, TOK_BLK], F16)
                xl_sb = xpool.tile([P, D_CHUNKS, TOK_BLK], F16)
                nc.sync.dma_start(out=xh_sb, in_=xhP[blk, :, :, :])
                nc.sync.dma_start(out=xl_sb, in_=xlP[blk, :, :, :])
                nc.tensor.matmul(sink_ps, lhsT=xh_sb[:, 0, 0:1], rhs=xh_sb[:, 0, 0:1])
                nc.tensor.matmul(sink_ps, lhsT=xl_sb[:, 0, 0:1], rhs=xl_sb[:, 0, 0:1])

                # logits.T: main term -> PSUM partitions 0:64, correction
                # terms (x2048) -> 64:128; the two column tiles run
                # concurrently on the PE array.
                lgT_ps = psT.tile([P, TOK_BLK], F32)
                for c in range(D_CHUNKS):
                    nc.tensor.matmul(
                        lgT_ps[0:E, :], lhsT=wh_sb[:, c, :], rhs=xh_sb[:, c, :],
                        start=(c == 0), stop=(c == D_CHUNKS - 1),
                        skip_group_check=True,
                    )
                    nc.tensor.matmul(
                        lgT_ps[E : 2 * E, :], lhsT=wl_sb[:, c, :], rhs=xh_sb[:, c, :],
                        start=(c == 0), stop=False, skip_group_check=True,
                    )
                    nc.tensor.matmul(
                        lgT_ps[E : 2 * E, :], lhsT=wh_sb[:, c, :], rhs=xl_sb[:, c, :],
                        start=False, stop=(c == D_CHUNKS - 1), skip_group_check=True,
                    )

                # lgT = correction/2048 + main   [64, 512] fp32 in SBUF
                # (DVE has one PSUM read port: ACT moves the main half to
                # SBUF, DVE adds the scaled correction from PSUM onto it.)
                lgT_sb = stage.tile([E, TOK_BLK], F32)
                nc.scalar.copy(lgT_sb, lgT_ps[0:E, :])
                nc.vector.scalar_tensor_tensor(
                    out=lgT_sb, in0=lgT_ps[E : 2 * E, :], scalar=1.0 / SPLIT_SCALE,
                    in1=lgT_sb, op0=MUL, op1=ADD,
                )

                # transpose back to [128 tok, 64 exp] tiles (PSUM, one bank)
                lg_ps = psL.tile([P, SUBS, E], F32)
                for sub in range(SUBS):
                    nc.tensor.matmul(
                        lg_ps[:, sub, :], lhsT=lgT_sb[:, sub * P : (sub + 1) * P],
                        rhs=ident, is_transpose=True, skip_group_check=True,
                    )

                mask_blk = maskf_sb[:, blk * SUBS : (blk + 1) * SUBS]  # [128, 4]

                # masked logits PSUM -> SBUF (also the DMA staging buffer)
                lg_sb = stage.tile([P, SUBS, E], F32)
                nc.vector.tensor_mul(lg_sb, lg_ps, _bcast(mask_blk, E))
                nc.sync.dma_start(out=logits_d[blk, :, :, :], in_=lg_sb)

                # exps (unmasked is fine: masked rows are overridden later)
                exp_sb = stage.tile([P, SUBS, E], F32)
                nc.scalar.activation(
                    out=exp_sb, in_=lg_ps, func=mybir.ActivationFunctionType.Exp
                )

                sums = small.tile([P, SUBS, 1], F32)
                nc.vector.reduce_sum(sums, exp_sb, axis=mybir.AxisListType.X)
                r_t = small.tile([P, SUBS], F32)
                nc.vector.reciprocal(r_t, sums[:, :, 0])
                r2_t = small.tile([P, SUBS], F32)
                nc.vector.tensor_mul(r2_t, r_t, mask_blk)
                pr_sb = stage.tile([P, SUBS, E], F32)
                nc.vector.tensor_mul(pr_sb, exp_sb, _bcast(r2_t[:, :], E))
                nc.sync.dma_start(out=probs_d[blk, :, :, :], in_=pr_sb)

                for sub in range(SUBS):
                    i = blk * SUBS + sub
                    nc.vector.max(out=top8[:, i, :], in_=exp_sb[:, sub, :])
                    nc.vector.max_index(
                        out=idx8[:, i, :], in_max=top8[:, i, :],
                        in_values=exp_sb[:, sub, :],
                    )

            # ---- tail: renormalized top-2 weights + masked indices ----
            s_t = accs.tile([P, n_tiles], F32)
            nc.vector.tensor_add(s_t, top8[:, :, 0], top8[:, :, 1])
            rs_t = accs.tile([P, n_tiles], F32)
            nc.vector.reciprocal(rs_t, s_t)
            nc.vector.tensor_mul(rs_t, rs_t, maskf_sb)
            w_out = accs.tile([P, n_tiles, TOP_K], F32)
            for k in range(TOP_K):
                nc.vector.tensor_mul(w_out[:, :, k], top8[:, :, k], rs_t)
            nc.sync.dma_start(out=weights_d[:, :, :], in_=w_out)

            # indices: (idx + 1) * mask - 1  (exact in fp32)
            idxf = accs.tile([P, n_tiles, TOP_K], F32)
            nc.vector.tensor_copy(idxf, idx8[:, :, 0:TOP_K])
            for k in range(TOP_K):
                nc.vector.tensor_scalar_add(idxf[:, :, k], idxf[:, :, k], 1.0)
                nc.vector.tensor_mul(idxf[:, :, k], idxf[:, :, k], maskf_sb)
                nc.vector.tensor_scalar_add(idxf[:, :, k], idxf[:, :, k], -1.0)
            idxi = accs.tile([P, n_tiles, TOP_K], I32)
            nc.vector.tensor_copy(idxi, idxf)
            nc.sync.dma_start(out=indices_d[:, :, :], in_=idxi)

    # Legalization (splits >1-wait instructions into event-semaphore ops,
    # moves matmul waits to ldweights) — required by walrus codegen.
    nc.compile()
    return nc


_NC_CACHE: dict[int, bacc.Bacc] = {}


def _get_nc(t_core: int = T) -> bacc.Bacc:
    if t_core not in _NC_CACHE:
        _NC_CACHE[t_core] = build_moe_router(t_core)
    return _NC_CACHE[t_core]


def _split16(a: np.ndarray):
    hi = a.astype(np.float16)
    lo = ((a - hi.astype(np.float32)) * SPLIT_SCALE).astype(np.float16)
    return hi, lo


def _pack_x(xh: np.ndarray, t_core: int) -> np.ndarray:
    """[T, D] fp16 -> [n_blk, 128p, 8c, 512t] matching the SBUF tiles."""
    n_blk = t_core // TOK_BLK
    return np.ascontiguousarray(
        xh.reshape(n_blk, TOK_BLK, D_CHUNKS, P).transpose(0, 3, 2, 1)
    )


def make_in_maps(x: np.ndarray, x_mask: np.ndarray, W: np.ndarray):
    """Shard full inputs into per-core input maps (host-side layout prep)."""
    t_core = x.shape[1]
    n_tiles = t_core // P
    wh, wl = _split16(np.asarray(W, dtype=np.float32))
    whT = np.ascontiguousarray(wh.T)
    wlT = np.ascontiguousarray(wl.T)
    in_maps = []
    for b in range(x.shape[0]):
        xh, xl = _split16(np.asarray(x[b], dtype=np.float32))
        mf = np.ascontiguousarray(
            np.asarray(x_mask[b], dtype=np.float32).reshape(n_tiles, P).T
        )
        in_maps.append(
            {
                "xhP": _pack_x(xh, t_core),
                "xlP": _pack_x(xl, t_core),
                "whT": whT,
                "wlT": wlT,
                "maskf": mf,
            }
        )
    return in_maps


def _unpack_te(a: np.ndarray, t_core: int) -> np.ndarray:
    """[n_blk, 128p, 4sub, E] -> [T, E]."""
    return np.ascontiguousarray(
        a.transpose(0, 2, 1, 3).reshape(t_core, a.shape[-1])
    )


def _unpack_tk(a: np.ndarray, t_core: int) -> np.ndarray:
    """[128p, n_tiles, K] -> [T, K]."""
    return np.ascontiguousarray(a.transpose(1, 0, 2).reshape(t_core, a.shape[-1]))


def run_kernel(x, x_mask, W, trace: bool = False, trace_kwargs: dict | None = None):
    """Run on hardware; returns (outputs_tuple, BassKernelResults)."""
    from concourse.bass_utils import run_bass_kernel_spmd

    x = np.asarray(x)
    x_mask = np.asarray(x_mask)
    W = np.asarray(W)
    n_cores, t_core = x.shape[0], x.shape[1]
    nc = _get_nc(t_core)
    in_maps = make_in_maps(x, x_mask, W)
    res = run_bass_kernel_spmd(
        nc,
        in_maps,
        core_ids=list(range(n_cores)),
        trace=trace,
        **(trace_kwargs or {}),
    )
    ew = np.stack([_unpack_tk(res.results[b]["weights"], t_core) for b in range(n_cores)])
    ei = np.stack([_unpack_tk(res.results[b]["indices"], t_core) for b in range(n_cores)])
    rl = np.stack([_unpack_te(res.results[b]["logits"], t_core) for b in range(n_cores)])
    rp = np.stack([_unpack_te(res.results[b]["probs"], t_core) for b in range(n_cores)])
    return (ew, ei, rl, rp), res


def kernel(**inputs):
    outs, _ = run_kernel(
        inputs["x"], inputs["x_mask"], inputs["W"],
        trace=os.environ.get("MOE_TRACE", "") == "1",
    )
    return outs


# revision 13
# speedup vs baseline: 2.3397x; 1.1092x over previous
"""MoE router kernel for Trainium2 (Bass/Tile), SPMD across 8 NeuronCores.

Problem: nn_MoERouter (B=8, T=4096, D=1024, E=64, TOP_K=2).

  router_logits = (x * mask) @ W.T * mask          # (B, T, E)
  router_probs  = softmax(router_logits) * mask
  expert_weights, expert_indices = top_k(probs, 2), renormalized, masked
  padded tokens get index -1

Sharding: data-parallel over the batch axis; core b handles x[b] (4096
tokens). W is tiny and replicated. No cross-core communication.

Matmul strategy (per core): plain fp32 matmul on TRN2 lowers to 2
half-rate passes (~8 ns/output-col measured) and float32r is only
~tf32-accurate (1.5e-4 — would flip near-tie expert indices). Instead we
use an error-compensated fp16 split computed on the host:

  x = xh + xls/2048,  W = Wh + Wls/2048   (xh/xls/Wh/Wls all fp16)
  logits = xh@Wh + (xh@Wls + xls@Wh)/2048    (drops xls@Wls ~ 2^-22)

Max logits error ~4e-6 (verified on the real inputs: 0/65536 index
flips), DMA volume unchanged (2+2 bytes/elem), and the matmuls run at
full 1 cycle/row rate.

Layouts: the contraction (d) must be on SBUF partitions and DMA
descriptors want long contiguous per-partition runs, so the host
pre-tiles x into the exact per-block SBUF layout
[n_blk, 128p, 8chunk, 512tok] (every DMA is a fully contiguous 1 MB
read, 8 KB per partition). logits/probs are likewise written in packed
per-block layout [n_blk, 128p, 4sub, 64e] (1 KB/partition runs) and
unpacked on the host.

Compute: W-stationary matmuls, N=512 tokens, out = logits.T [64, 512]
in PSUM. The main term accumulates in PSUM partitions 0:64 and both
correction terms in 64:128 (concurrent column-packed PE tiles), then
ACT moves the main half to SBUF and one DVE scalar_tensor_tensor adds
correction/2048 (DVE has a single PSUM read port). Four PE transposes
per block bring logits back to [128 tok, 64 exp], ACT does a batched
exp, DVE does softmax scaling + top-8 with indices
(InstMax/InstMaxIndex match jax.lax.top_k tie order).
"""

import os
import sys

import numpy as np

for _p in ("/opt/trn_rl_repo", "/opt/pypackages"):
    if _p not in sys.path and os.path.isdir(_p):
        sys.path.append(_p)

import concourse.bass as bass
import concourse.mybir as mybir
from concourse import bacc
from concourse.masks import make_identity
from concourse.tile import TileContext

F32 = mybir.dt.float32
F16 = mybir.dt.float16
I32 = mybir.dt.int32
U32 = mybir.dt.uint32

B, T, D, E, TOP_K = 8, 4096, 1024, 64, 2
N_CORES = 8
P = 128                    # SBUF partitions
D_CHUNKS = D // P          # 8 contraction chunks
TOK_BLK = 512              # tokens per block (matmul free dim)
SUBS = TOK_BLK // P        # 4 token tiles per block
SPLIT_SCALE = 2048.0       # 2^11 residual scale for the fp16 split


def _bcast(ap: bass.AP, n: int) -> bass.AP:
    """Append a step-0 dim of size n (free-dim broadcast for DVE reads)."""
    return bass.AP(tensor=ap.tensor, offset=ap.offset, ap=[*ap.ap, [0, n]])


def build_moe_router(t_core: int = T) -> bacc.Bacc:
    """Build the per-core Bass program. t_core tokens per core (mult of 512)."""
    assert t_core % TOK_BLK == 0
    n_blk = t_core // TOK_BLK
    n_tiles = t_core // P

    nc = bacc.Bacc("TRN2", target_bir_lowering=False, debug=False)

    xP = nc.dram_tensor("xP", [n_blk, P, 2, D_CHUNKS, TOK_BLK], F16, kind="ExternalInput")
    whT = nc.dram_tensor("whT", [D, E], F16, kind="ExternalInput")
    wlT = nc.dram_tensor("wlT", [D, E], F16, kind="ExternalInput")
    maskf = nc.dram_tensor("maskf", [P, n_tiles], F32, kind="ExternalInput")
    logits_d = nc.dram_tensor("logits", [n_blk, P, SUBS, E], F32, kind="ExternalOutput")
    probs_d = nc.dram_tensor("probs", [n_blk, P, SUBS, E], F32, kind="ExternalOutput")
    weights_d = nc.dram_tensor("weights", [P, n_tiles, TOP_K], F32, kind="ExternalOutput")
    indices_d = nc.dram_tensor("indices", [P, n_tiles, TOP_K], I32, kind="ExternalOutput")

    whT_t = whT.rearrange("(c p) e -> p c e", p=P)        # [128, 8, 64]
    wlT_t = wlT.rearrange("(c p) e -> p c e", p=P)

    MUL = mybir.AluOpType.mult
    ADD = mybir.AluOpType.add

    with TileContext(nc) as tc:
        with (
            tc.tile_pool(name="xpool", bufs=4) as xpool,
            tc.tile_pool(name="consts", bufs=1) as consts,
            tc.tile_pool(name="psT", bufs=3, space="PSUM") as psT,
            tc.tile_pool(name="psL", bufs=2, space="PSUM") as psL,
            tc.tile_pool(name="psink", bufs=1, space="PSUM") as psink,
            tc.tile_pool(name="stage", bufs=3) as stage,
            tc.tile_pool(name="small", bufs=6) as small,
            tc.tile_pool(name="accs", bufs=1) as accs,
        ):
            wh_sb = consts.tile([P, D_CHUNKS, E], F16)
            wl_sb = consts.tile([P, D_CHUNKS, E], F16)
            nc.sync.dma_start(out=wh_sb, in_=whT_t)
            nc.sync.dma_start(out=wl_sb, in_=wlT_t)
            maskf_sb = consts.tile([P, n_tiles], F32)
            nc.sync.dma_start(out=maskf_sb, in_=maskf[:, :])
            ident = consts.tile([E, E], F32)
            make_identity(nc, ident)

            top8 = accs.tile([P, n_tiles, 8], F32)
            idx8 = accs.tile([P, n_tiles, 8], U32)

            # PE sink matmuls absorb DMA-completion waits so real matmuls
            # carry at most one wait (walrus limit on Matmult sync waits).
            sink_ps = psink.tile([1, 1], F32)
            nc.tensor.matmul(sink_ps, lhsT=wh_sb[:, 0, 0:1], rhs=wh_sb[:, 0, 0:1])
            nc.tensor.matmul(sink_ps, lhsT=wl_sb[:, 0, 0:1], rhs=wl_sb[:, 0, 0:1])

            for blk in range(n_blk):
                x_sb = xpool.tile([P, 2, D_CHUNKS, TOK_BLK], F16)
                nc.sync.dma_start(out=x_sb, in_=xP[blk, :, :, :, :])
                xh_sb = x_sb[:, 0]
                xl_sb = x_sb[:, 1]
                nc.tensor.matmul(sink_ps, lhsT=x_sb[:, 0, 0, 0:1], rhs=x_sb[:, 0, 0, 0:1])

                # logits.T: main term -> PSUM partitions 0:64, correction
                # terms (x2048) -> 64:128; the two column tiles run
                # concurrently on the PE array.
                lgT_ps = psT.tile([P, TOK_BLK], F32)
                for c in range(D_CHUNKS):
                    nc.tensor.matmul(
                        lgT_ps[0:E, :], lhsT=wh_sb[:, c, :], rhs=xh_sb[:, c, :],
                        start=(c == 0), stop=(c == D_CHUNKS - 1),
                        skip_group_check=True,
                    )
                    nc.tensor.matmul(
                        lgT_ps[E : 2 * E, :], lhsT=wl_sb[:, c, :], rhs=xh_sb[:, c, :],
                        start=(c == 0), stop=False, skip_group_check=True,
                    )
                    nc.tensor.matmul(
                        lgT_ps[E : 2 * E, :], lhsT=wh_sb[:, c, :], rhs=xl_sb[:, c, :],
                        start=False, stop=(c == D_CHUNKS - 1), skip_group_check=True,
                    )

                # lgT = correction/2048 + main   [64, 512] fp32 in SBUF
                # (DVE has one PSUM read port: ACT moves the main half to
                # SBUF, DVE adds the scaled correction from PSUM onto it.)
                lgT_sb = stage.tile([E, TOK_BLK], F32)
                nc.scalar.copy(lgT_sb, lgT_ps[0:E, :])
                nc.vector.scalar_tensor_tensor(
                    out=lgT_sb, in0=lgT_ps[E : 2 * E, :], scalar=1.0 / SPLIT_SCALE,
                    in1=lgT_sb, op0=MUL, op1=ADD,
                )

                # transpose back to [128 tok, 64 exp] tiles (PSUM, one bank)
                lg_ps = psL.tile([P, SUBS, E], F32)
                for sub in range(SUBS):
                    nc.tensor.matmul(
                        lg_ps[:, sub, :], lhsT=lgT_sb[:, sub * P : (sub + 1) * P],
                        rhs=ident, is_transpose=True, skip_group_check=True,
                    )

                mask_blk = maskf_sb[:, blk * SUBS : (blk + 1) * SUBS]  # [128, 4]

                # masked logits PSUM -> SBUF (also the DMA staging buffer)
                lg_sb = stage.tile([P, SUBS, E], F32)
                nc.vector.tensor_mul(lg_sb, lg_ps, _bcast(mask_blk, E))
                nc.scalar.dma_start(out=logits_d[blk, :, :, :], in_=lg_sb)

                # exps (unmasked is fine: masked rows are overridden later)
                exp_sb = stage.tile([P, SUBS, E], F32)
                nc.scalar.activation(
                    out=exp_sb, in_=lg_ps, func=mybir.ActivationFunctionType.Exp
                )

                sums = small.tile([P, SUBS, 1], F32)
                nc.vector.reduce_sum(sums, exp_sb, axis=mybir.AxisListType.X)
                r_t = small.tile([P, SUBS], F32)
                nc.vector.reciprocal(r_t, sums[:, :, 0])
                r2_t = small.tile([P, SUBS], F32)
                nc.vector.tensor_mul(r2_t, r_t, mask_blk)
                pr_sb = stage.tile([P, SUBS, E], F32)
                nc.vector.tensor_mul(pr_sb, exp_sb, _bcast(r2_t[:, :], E))
                nc.scalar.dma_start(out=probs_d[blk, :, :, :], in_=pr_sb)

                for sub in range(SUBS):
                    i = blk * SUBS + sub
                    nc.vector.max(out=top8[:, i, :], in_=exp_sb[:, sub, :])
                    nc.vector.max_index(
                        out=idx8[:, i, :], in_max=top8[:, i, :],
                        in_values=exp_sb[:, sub, :],
                    )

            # ---- tail: renormalized top-2 weights + masked indices ----
            s_t = accs.tile([P, n_tiles], F32)
            nc.vector.tensor_add(s_t, top8[:, :, 0], top8[:, :, 1])
            rs_t = accs.tile([P, n_tiles], F32)
            nc.vector.reciprocal(rs_t, s_t)
            nc.vector.tensor_mul(rs_t, rs_t, maskf_sb)
            w_out = accs.tile([P, n_tiles, TOP_K], F32)
            for k in range(TOP_K):
                nc.vector.tensor_mul(w_out[:, :, k], top8[:, :, k], rs_t)
            nc.scalar.dma_start(out=weights_d[:, :, :], in_=w_out)

            # indices: (idx + 1) * mask - 1  (exact in fp32)
            idxf = accs.tile([P, n_tiles, TOP_K], F32)
            nc.vector.tensor_copy(idxf, idx8[:, :, 0:TOP_K])
            for k in range(TOP_K):
                nc.vector.tensor_scalar_add(idxf[:, :, k], idxf[:, :, k], 1.0)
                nc.vector.tensor_mul(idxf[:, :, k], idxf[:, :, k], maskf_sb)
                nc.vector.tensor_scalar_add(idxf[:, :, k], idxf[:, :, k], -1.0)
            idxi = accs.tile([P, n_tiles, TOP_K], I32)
            nc.vector.tensor_copy(idxi, idxf)
            nc.scalar.dma_start(out=indices_d[:, :, :], in_=idxi)

    # Legalization (splits >1-wait instructions into event-semaphore ops,
    # moves matmul waits to ldweights) — required by walrus codegen.
    nc.compile()
    return nc


_NC_CACHE: dict[int, bacc.Bacc] = {}


def _get_nc(t_core: int = T) -> bacc.Bacc:
    if t_core not in _NC_CACHE:
        _NC_CACHE[t_core] = build_moe_router(t_core)
    return _NC_CACHE[t_core]


def _split16(a: np.ndarray):
    hi = a.astype(np.float16)
    lo = ((a - hi.astype(np.float32)) * SPLIT_SCALE).astype(np.float16)
    return hi, lo


def _pack_x(xh: np.ndarray, xl: np.ndarray, t_core: int) -> np.ndarray:
    """2x [T, D] fp16 -> [n_blk, 128p, 2, 8c, 512t] matching the SBUF tiles."""
    n_blk = t_core // TOK_BLK
    both = np.stack([xh, xl], axis=0)  # [2, T, D]
    return np.ascontiguousarray(
        both.reshape(2, n_blk, TOK_BLK, D_CHUNKS, P).transpose(1, 4, 0, 3, 2)
    )


def make_in_maps(x: np.ndarray, x_mask: np.ndarray, W: np.ndarray):
    """Shard full inputs into per-core input maps (host-side layout prep)."""
    t_core = x.shape[1]
    n_tiles = t_core // P
    wh, wl = _split16(np.asarray(W, dtype=np.float32))
    whT = np.ascontiguousarray(wh.T)
    wlT = np.ascontiguousarray(wl.T)
    in_maps = []
    for b in range(x.shape[0]):
        xh, xl = _split16(np.asarray(x[b], dtype=np.float32))
        mf = np.ascontiguousarray(
            np.asarray(x_mask[b], dtype=np.float32).reshape(n_tiles, P).T
        )
        in_maps.append(
            {
                "xP": _pack_x(xh, xl, t_core),
                "whT": whT,
                "wlT": wlT,
                "maskf": mf,
            }
        )
    return in_maps


def _unpack_te(a: np.ndarray, t_core: int) -> np.ndarray:
    """[n_blk, 128p, 4sub, E] -> [T, E]."""
    return np.ascontiguousarray(
        a.transpose(0, 2, 1, 3).reshape(t_core, a.shape[-1])
    )


def _unpack_tk(a: np.ndarray, t_core: int) -> np.ndarray:
    """[128p, n_tiles, K] -> [T, K]."""
    return np.ascontiguousarray(a.transpose(1, 0, 2).reshape(t_core, a.shape[-1]))


def run_kernel(x, x_mask, W, trace: bool = False, trace_kwargs: dict | None = None):
    """Run on hardware; returns (outputs_tuple, BassKernelResults)."""
    from concourse.bass_utils import run_bass_kernel_spmd

    x = np.asarray(x)
    x_mask = np.asarray(x_mask)
    W = np.asarray(W)
    n_cores, t_core = x.shape[0], x.shape[1]
    nc = _get_nc(t_core)
    in_maps = make_in_maps(x, x_mask, W)
    res = run_bass_kernel_spmd(
        nc,
        in_maps,
        core_ids=list(range(n_cores)),
        trace=trace,
        **(trace_kwargs or {}),
    )
    ew = np.stack([_unpack_tk(res.results[b]["weights"], t_core) for b in range(n_cores)])
    ei = np.stack([_unpack_tk(res.results[b]["indices"], t_core) for b in range(n_cores)])
    rl = np.stack([_unpack_te(res.results[b]["logits"], t_core) for b in range(n_cores)])
    rp = np.stack([_unpack_te(res.results[b]["probs"], t_core) for b in range(n_cores)])
    return (ew, ei, rl, rp), res


def kernel(**inputs):
    outs, _ = run_kernel(
        inputs["x"], inputs["x_mask"], inputs["W"],
        trace=os.environ.get("MOE_TRACE", "") == "1",
    )
    return outs
